# revision 16
# baseline (speedup 1.0000x reference)
"""Trainium2 Bass kernel for nn_Cascade_CNN_RNN (cascade CNN -> MGU RNN).

Data-parallel over batch across 8 NeuronCores. Per core (shard B=256):
  - quantize(x) on DVE (magic-constant round-half-even, exact vs jnp.round)
  - conv1 as banded spatial-operator matmuls -> a1 in 20 row-blocks
    [88 = (ci_half(8) x col(11)), block r = (row y, half h)]
  - conv2 as row-blocked banded matmuls (3-row neighborhoods, 6 shared
    Toeplitz lhsT), clip(0,1) epilogues -> F [128, 30, Sc]
  - fc3 (30 K-chunks) + gi = a3 @ w_ih.T hoisted over all 10 windows
  - sequential 10-step MGU on [64, 256] + fc5
All matmul-facing tensors are float32r (full-speed PE, ~1e-4 rel err).
"""

import numpy as np

import concourse.bass as bass
import concourse.mybir as mybir
import concourse.tile as tile
from concourse import bacc
from concourse.bass_utils import run_bass_kernel_spmd

F32 = mybir.dt.float32
F32R = mybir.dt.float32r
MAGIC = 12582912.0  # 1.5 * 2**23: fp32 round-to-nearest-even integer trick
INV_SCALE = 0.0078125  # 1/128

T, HH, WW = 10, 10, 11  # windows, height, width
SP = HH * WW  # 110 input spatial positions
CH1, CH2 = 16, 32
NCLS = 7
HID = 64


# ---------------------------------------------------------------- host packing
def _pack_weights(conv1_w, conv2_w, fc3_w, w_ih, w_hh, fc5_w):
    # conv1 operator: W1T[k=(yy*11+xx), r=(y*2+h), p=(c*11+x)]
    w1t = np.zeros((128, 20, 88), np.float32)
    for y in range(HH):
        for h in range(2):
            r = y * 2 + h
            for c in range(8):
                ci = h * 8 + c
                for x in range(WW):
                    p = c * WW + x
                    for ky in range(3):
                        yy = y + ky - 1
                        if not (0 <= yy < HH):
                            continue
                        for kx in range(3):
                            xx = x + kx - 1
                            if not (0 <= xx < WW):
                                continue
                            w1t[yy * WW + xx, r, p] = conv1_w[ci, 0, ky, kx]
    w1t = w1t.reshape(128, 20 * 88)

    # conv2 Toeplitz bands: W2T[dh=(dr*2+h), p=(c*11+x), m=(co2*11+j)] pad M->384
    w2t = np.zeros((6, 88, 384), np.float32)
    for dr in range(3):
        for h in range(2):
            dh = dr * 2 + h
            for c in range(8):
                ci = h * 8 + c
                for x in range(WW):
                    p = c * WW + x
                    for co in range(CH2):
                        for j in range(WW):
                            kx = x - j + 1
                            if 0 <= kx < 3:
                                w2t[dh, p, co * WW + j] = conv2_w[co, ci, dr, kx]

    # fc3: FC3T[k=(i*3+m), p, fc] = fc3_w[fc, co2*110 + i*11 + j], mm=m*128+p
    fc3t = np.zeros((30, 128, 256), np.float32)
    for i in range(HH):
        for m in range(3):
            mm = np.arange(128) + m * 128
            valid = mm < 352
            co2 = mm[valid] // WW
            j = mm[valid] % WW
            g = co2 * SP + i * WW + j
            fc3t[i * 3 + m, valid, :] = fc3_w[:, g].T

    wiht = np.ascontiguousarray(
        w_ih.reshape(2 * HID, 2, 128).transpose(1, 2, 0)
    )  # [mf, p, gate]
    whht = np.ascontiguousarray(w_hh.T)  # [64, 128]
    fc5t = np.ascontiguousarray(fc5_w.T)  # [64, 7]
    return w1t, w2t.reshape(6 * 88, 384), fc3t.reshape(30 * 128, 256), wiht.reshape(2 * 128, 128), whht, fc5t


def _pack_x(x_shard):
    # [BS, T, HH, WW] -> [110, S] with s = t*BS + b
    BS = x_shard.shape[0]
    xt = x_shard.transpose(1, 0, 2, 3).reshape(T * BS, SP).T
    return np.ascontiguousarray(xt)


# ---------------------------------------------------------------- bass builder
def build_nc(BS=256, Sc=256):
    S = T * BS
    assert S % Sc == 0
    NCHUNK = S // Sc
    nc = bacc.Bacc()

    xt_d = nc.declare_dram_parameter("xt", [SP, S], F32, isOutput=False)
    w1_d = nc.declare_dram_parameter("w1t", [128, 20 * 88], F32R, isOutput=False)
    w2_d = nc.declare_dram_parameter("w2t", [6 * 88, 384], F32R, isOutput=False)
    f3_d = nc.declare_dram_parameter("fc3t", [30 * 128, 256], F32R, isOutput=False)
    wi_d = nc.declare_dram_parameter("wiht", [2 * 128, 128], F32R, isOutput=False)
    wh_d = nc.declare_dram_parameter("whht", [HID, 128], F32R, isOutput=False)
    f5_d = nc.declare_dram_parameter("fc5t", [HID, NCLS], F32R, isOutput=False)
    out_d = nc.declare_dram_parameter("out", [NCLS, BS], F32, isOutput=True)

    MX = mybir.AluOpType.max
    MN = mybir.AluOpType.min
    AD = mybir.AluOpType.add
    SU = mybir.AluOpType.subtract
    MU = mybir.AluOpType.mult

    with tile.TileContext(nc) as tc:
        with (
            tc.tile_pool(name="static", bufs=1) as st,
            tc.tile_pool(name="a1p", bufs=2) as a1p,
            tc.tile_pool(name="fp", bufs=1) as fp,
            tc.tile_pool(name="a3p", bufs=2) as a3p,
            tc.tile_pool(name="rp", bufs=2) as rp,
            tc.tile_pool(name="xp", bufs=2) as xp,
            tc.tile_pool(name="c1ps", bufs=2, space="PSUM") as c1ps,
            tc.tile_pool(name="c2ps", bufs=2, space="PSUM") as c2ps,
            tc.tile_pool(name="f3ps", bufs=2, space="PSUM") as f3ps,
            tc.tile_pool(name="rps", bufs=2, space="PSUM") as rps,
        ):
            # ---- static loads
            W1 = st.tile([128, 20, 88], F32R)
            nc.sync.dma_start(W1[:], w1_d.ap().rearrange("k (r p) -> k r p", r=20))
            W2 = st.tile([88, 6, 384], F32R)
            nc.sync.dma_start(
                W2[:], w2_d.ap().rearrange("(d p) m -> p d m", d=6)
            )
            FC3 = st.tile([128, 30, 256], F32R)
            nc.sync.dma_start(
                FC3[:], f3_d.ap().rearrange("(k p) f -> p k f", k=30)
            )
            WIH = st.tile([128, 2, 128], F32R)
            nc.sync.dma_start(WIH[:], wi_d.ap().rearrange("(m p) g -> p m g", m=2))
            WHH = st.tile([HID, 128], F32R)
            nc.sync.dma_start(WHH[:], wh_d.ap())
            FC5 = st.tile([HID, NCLS], F32R)
            nc.sync.dma_start(FC5[:], f5_d.ap())

            GIF = st.tile([HID, S], F32)  # gi forget-gate half
            GIN = st.tile([HID, S], F32)  # gi new-gate half

            # ---- batched encoder: conv1 -> conv2 -> fc3 -> gi, per s-chunk
            for u in range(NCHUNK):
                sl = bass.ts(u, Sc)
                # x load + quantize for this chunk ([110, Sc] layout)
                XIN = xp.tile([SP, Sc], F32, name="XIN")
                nc.sync.dma_start(XIN[:], xt_d.ap()[:, sl])
                nc.vector.tensor_scalar(XIN[:], XIN[:], 128.0, -128.0, MU, MX)
                nc.vector.tensor_scalar(XIN[:], XIN[:], 128.0, MAGIC, MN, AD)
                XQ = xp.tile([SP, Sc], F32R, name="XQ")
                nc.vector.tensor_scalar(XQ[:], XIN[:], MAGIC, INV_SCALE, SU, MU)

                A1 = a1p.tile([88, 20, Sc], F32R, name="A1")
                for r in range(20):
                    ps1 = c1ps.tile([88, Sc], F32, name="ps1")
                    nc.tensor.matmul(
                        ps1[:], W1[:SP, r, :], XQ[:], start=True, stop=True
                    )
                    nc.vector.tensor_scalar(A1[:, r, :], ps1[:], 0.0, 1.0, MX, MN)

                F = fp.tile([128, 30, Sc], F32R, name="F")
                for i in range(HH):
                    rows = [dr for dr in range(3) if 0 <= i + dr - 1 < HH]
                    for m in range(3):
                        ps2 = c2ps.tile([128, Sc], F32, name="ps2")
                        nmm = len(rows) * 2
                        q = 0
                        for dr in rows:
                            y = i + dr - 1
                            for h in range(2):
                                nc.tensor.matmul(
                                    ps2[:],
                                    W2[:, dr * 2 + h, bass.ts(m, 128)],
                                    A1[:, y * 2 + h, :],
                                    start=(q == 0),
                                    stop=(q == nmm - 1),
                                )
                                q += 1
                        nc.vector.tensor_scalar(
                            F[:, i * 3 + m, :], ps2[:], 0.0, 1.0, MX, MN
                        )

                A3 = a3p.tile([128, 2, Sc], F32R, name="A3")
                for mf in range(2):
                    ps3 = f3ps.tile([128, Sc], F32, name="ps3")
                    for k in range(30):
                        nc.tensor.matmul(
                            ps3[:],
                            FC3[:, k, bass.ts(mf, 128)],
                            F[:, k, :],
                            start=(k == 0),
                            stop=(k == 29),
                        )
                    nc.vector.tensor_scalar(A3[:, mf, :], ps3[:], 0.0, 1.0, MX, MN)

                psgf = f3ps.tile([HID, Sc], F32, name="psgf", tag="ps3")
                for mf in range(2):
                    nc.tensor.matmul(
                        psgf[:], WIH[:, mf, :HID], A3[:, mf, :],
                        start=(mf == 0), stop=(mf == 1),
                    )
                nc.vector.tensor_copy(GIF[:, sl], psgf[:])
                psgn = f3ps.tile([HID, Sc], F32, name="psgn", tag="ps3")
                for mf in range(2):
                    nc.tensor.matmul(
                        psgn[:], WIH[:, mf, HID:128], A3[:, mf, :],
                        start=(mf == 0), stop=(mf == 1),
                    )
                nc.vector.tensor_copy(GIN[:, sl], psgn[:])

            # ---- recurrence: 10 MGU steps on [64, BS]
            H = st.tile([HID, BS], F32)
            HF = st.tile([HID, BS], F32R)  # final hidden, f32r for fc5
            hf = H[:]
            nc.vector.memset(hf, 0.0)
            for t in range(T):
                ts_sl = bass.ts(t, BS)
                HQ = rp.tile([HID, BS], F32R, name="HQ")
                qtmp = rp.tile([HID, BS], F32, name="qtmp")
                nc.vector.tensor_scalar(qtmp[:], hf, 128.0, -128.0, MU, MX)
                nc.vector.tensor_scalar(qtmp[:], qtmp[:], 128.0, MAGIC, MN, AD)
                nc.vector.tensor_scalar(HQ[:], qtmp[:], MAGIC, INV_SCALE, SU, MU)
                hqf = HQ[:].bitcast(F32)

                psf = rps.tile([HID, BS], F32, name="psf", tag="psr")
                nc.tensor.matmul(psf[:], WHH[:, :HID], HQ[:], start=True, stop=True)
                psn = rps.tile([HID, BS], F32, name="psn", tag="psr")
                nc.tensor.matmul(psn[:], WHH[:, HID:128], HQ[:], start=True, stop=True)

                fg = rp.tile([HID, BS], F32, name="fg")
                nc.vector.tensor_tensor(fg[:], GIF[:, ts_sl], psf[:], AD)
                nc.vector.tensor_scalar(fg[:], fg[:], 0.5, 0.5, MU, AD)
                nc.vector.tensor_scalar(fg[:], fg[:], 0.0, 1.0, MX, MN)

                ng = rp.tile([HID, BS], F32, name="ng")
                nc.vector.tensor_tensor(ng[:], fg[:], psn[:], MU)
                nc.vector.tensor_tensor(ng[:], ng[:], GIN[:, ts_sl], AD)
                nc.vector.tensor_scalar(ng[:], ng[:], -1.0, 1.0, MX, MN)

                fgm = rp.tile([HID, BS], F32, name="fgm")
                nc.vector.tensor_scalar(fgm[:], fg[:], -1.0, 1.0, MU, AD)
                nc.vector.tensor_tensor(fgm[:], fgm[:], ng[:], MU)
                nc.vector.tensor_tensor(fg[:], fg[:], hqf, MU)
                # last step writes the f32r tile so the fc5 matmul
                # sees an f32r producer (walrus verifier requirement)
                nc.vector.tensor_tensor(HF[:] if t == T - 1 else hf, fgm[:], fg[:], AD)

            pso = rps.tile([NCLS, BS], F32, name="pso", tag="psr")
            nc.tensor.matmul(pso[:], FC5[:], HF[:], start=True, stop=True)
            OUTS = rp.tile([NCLS, BS], F32, name="OUTS")
            nc.vector.tensor_copy(OUTS[:], pso[:])
            nc.sync.dma_start(out_d.ap(), OUTS[:])

    nc.compile()
    return nc


# ---------------------------------------------------------------- entry point
def kernel(**inputs):
    x = np.asarray(inputs["x"], np.float32)
    packs = _pack_weights(
        np.asarray(inputs["conv1_w"], np.float32),
        np.asarray(inputs["conv2_w"], np.float32),
        np.asarray(inputs["fc3_w"], np.float32),
        np.asarray(inputs["w_ih"], np.float32),
        np.asarray(inputs["w_hh"], np.float32),
        np.asarray(inputs["fc5_w"], np.float32),
    )
    w1t, w2t, fc3t, wiht, whht, fc5t = packs
    NCORES = 8
    B = x.shape[0]
    BS = B // NCORES

    nc = build_nc(BS=BS, Sc=256)
    in_maps = []
    for c in range(NCORES):
        in_maps.append(
            {
                "xt": _pack_x(x[c * BS : (c + 1) * BS]),
                "w1t": w1t,
                "w2t": w2t,
                "fc3t": fc3t,
                "wiht": wiht,
                "whht": whht,
                "fc5t": fc5t,
            }
        )
    res = run_bass_kernel_spmd(nc, in_maps, core_ids=list(range(NCORES)))
    out = np.concatenate([res.results[c]["out"].T for c in range(NCORES)], axis=0)
    return np.ascontiguousarray(out, np.float32)


if __name__ == "__main__":
    rng = np.random.default_rng(0)
    ins = {
        "x": rng.standard_normal((2048, T, HH, WW), np.float32) * 0.5,
        "conv1_w": rng.standard_normal((CH1, 1, 3, 3), np.float32) * 0.1,
        "conv2_w": rng.standard_normal((CH2, CH1, 3, 3), np.float32) * 0.1,
        "fc3_w": rng.standard_normal((256, 3520), np.float32) * 0.1,
        "w_ih": rng.standard_normal((128, 256), np.float32) * 0.1,
        "w_hh": rng.standard_normal((128, HID), np.float32) * 0.1,
        "fc5_w": rng.standard_normal((NCLS, HID), np.float32) * 0.1,
    }
    out = kernel(**ins)
    print(out.shape, out.dtype, np.abs(out).mean())
